# revision 4
# baseline (speedup 1.0000x reference)
"""AttentionDecoder Trainium2 kernel: 8-way model-parallel LSTM+attention decoder.

Strategy:
  - Weights sharded 8 ways over the gate/hidden dims, SBUF-resident.
  - Activations feature-major [feat, batch]; matmuls are activation-stationary
    (lhsT = activation [K=feat, M=batch], rhs = weight.T [K=feat, N=out_feats]).
  - Per timestep: 3 AllGathers (h0, h1, o) across the 8 cores.
  - Attention refactored: M1T[b] = (h_enc[b] @ W1).T and M2[b] = h_enc[b] @ W2v.T
    are precomputed (hoists h_enc out of the sequential loop), so per step
    scores[b] = M1T[b].T @ h1[:,b] + c1[b] and
    z[b] = a[b] @ M2[b] + W2h @ h1[:,b] + b2,  o = tanh(z).
  - Per-core batch shard for attention: core k owns batch 8k..8k+7.
"""

import os
import warnings

warnings.filterwarnings("ignore")

import numpy as np

VOCAB, E, H, L, B, T, S, V = 32000, 512, 1024, 2, 64, 64, 128, 1024
NCORES = 8
P = 128
BG = B // NCORES  # 8 batch per core for attention
HC = H // NCORES  # 128 hidden feats per core
GC = 4 * HC  # 512 gate rows per core

_COMPILED = None


def _build(n_steps: int):
    import concourse.bass as bass
    import concourse.bacc as bacc
    import concourse.mybir as mybir
    import concourse.tile as tile
    from concourse import masks

    fp32 = mybir.dt.float32
    AF = mybir.ActivationFunctionType
    AX = mybir.AxisListType

    nc = bacc.Bacc(
        "TRN2",
        target_bir_lowering=False,
        debug=False,
        num_devices=NCORES,
        monotonic_sem_count=7,
    )
    rsems = [nc.monotonic_semaphore(i).sem() for i in range(6)]  # h0e,h0o,h1e,h1o,oe,oo
    lsem = nc.monotonic_semaphore(6).sem()
    RD = [(0, d) for d in range(NCORES)]

    # ---- DRAM parameters (per-core data) ----
    d_xseq = nc.dram_tensor("xseq", [n_steps, P, 4, B], fp32, kind="ExternalInput")
    d_w0T = nc.dram_tensor("w0T", [16, P, GC], fp32, kind="ExternalInput")
    d_w1T = nc.dram_tensor("w1T", [16, P, GC], fp32, kind="ExternalInput")
    d_b0 = nc.dram_tensor("b0", [B, GC], fp32, kind="ExternalInput")
    d_b1 = nc.dram_tensor("b1", [B, GC], fp32, kind="ExternalInput")
    d_m1t = nc.dram_tensor("m1t", [BG, 8, P, S], fp32, kind="ExternalInput")
    d_c1t = nc.dram_tensor("c1t", [2, P, S], fp32, kind="ExternalInput")
    d_m2s = nc.dram_tensor("m2s", [BG, P, E], fp32, kind="ExternalInput")
    d_w2hT = nc.dram_tensor("w2hT", [8, P, E], fp32, kind="ExternalInput")
    d_b2 = nc.dram_tensor("b2", [BG, E], fp32, kind="ExternalInput")
    d_h0i = nc.dram_tensor("h0i", [P, 8, B], fp32, kind="ExternalInput")
    d_h1i = nc.dram_tensor("h1i", [P, 8, B], fp32, kind="ExternalInput")
    d_oi = nc.dram_tensor("oi", [P, 4, B], fp32, kind="ExternalInput")
    d_out = nc.dram_tensor("out", [n_steps, BG, E], fp32, kind="ExternalOutput")

    with tile.TileContext(nc) as tc:
        import contextlib

        ctx = contextlib.ExitStack()
        with ctx:
            wpool = ctx.enter_context(tc.tile_pool(name="weights", bufs=1))
            spool = ctx.enter_context(tc.tile_pool(name="state", bufs=1))
            xpool = ctx.enter_context(tc.tile_pool(name="x", bufs=2))
            tpool = ctx.enter_context(tc.tile_pool(name="tmp", bufs=2))
            ppool = ctx.enter_context(tc.tile_pool(name="psum", bufs=2, space="PSUM"))
            p1pool = ctx.enter_context(tc.tile_pool(name="psum1", bufs=1, space="PSUM"))
            dpool = ctx.enter_context(tc.tile_pool(name="dram", bufs=2, space="DRAM"))

            # ---- persistent SBUF tiles ----
            w0T = wpool.tile([P, 16, GC], fp32, tag="w0T")
            w1T = wpool.tile([P, 16, GC], fp32, tag="w1T")
            b0 = wpool.tile([B, GC], fp32, tag="b0")
            b1 = wpool.tile([B, GC], fp32, tag="b1")
            m1t = wpool.tile([P, BG, 8, S], fp32, tag="m1t")
            c1t = wpool.tile([P, 2, S], fp32, tag="c1t")
            m2s = wpool.tile([P, BG, E], fp32, tag="m2s")
            w2hT = wpool.tile([P, 8, E], fp32, tag="w2hT")
            b2 = wpool.tile([BG, E], fp32, tag="b2")
            ident = wpool.tile([P, P], fp32, tag="ident")

            h0f = [spool.tile([P, NCORES * B], fp32, tag=f"h0f{i}", name=f"h0f{i}") for i in range(2)]
            h1f = [spool.tile([P, NCORES * B], fp32, tag=f"h1f{i}", name=f"h1f{i}") for i in range(2)]
            of = [spool.tile([P, NCORES * 4 * BG], fp32, tag=f"of{i}", name=f"of{i}") for i in range(2)]
            c0 = spool.tile([B, HC], fp32, tag="c0")  # cell states, batch-major
            c1 = spool.tile([B, HC], fp32, tag="c1")
            h1my = spool.tile([P, 8, BG], fp32, tag="h1my")

            # ---- prologue loads ----
            for kt in range(16):
                nc.sync.dma_start(w0T[:, kt, :], d_w0T[kt])
                nc.sync.dma_start(w1T[:, kt, :], d_w1T[kt])
            nc.sync.dma_start(b0[:], d_b0[:])
            nc.sync.dma_start(b1[:], d_b1[:])
            for j in range(BG):
                for kt in range(8):
                    nc.sync.dma_start(m1t[:, j, kt, :], d_m1t[j, kt])
                nc.sync.dma_start(m2s[:, j, :], d_m2s[j])
            nc.sync.dma_start(c1t[:, 0, :], d_c1t[0])
            nc.sync.dma_start(c1t[:, 1, :], d_c1t[1])
            for kt in range(8):
                nc.sync.dma_start(w2hT[:, kt, :], d_w2hT[kt])
            nc.sync.dma_start(b2[:], d_b2[:])
            masks.make_identity(nc, ident[:])
            nc.sync.dma_start(h0f[1][:].rearrange("p (kc b) -> p kc b", kc=8), d_h0i[:])
            nc.sync.dma_start(h1f[1][:].rearrange("p (kc b) -> p kc b", kc=8), d_h1i[:])
            # of init: slot k holds o columns [c, j] for b-group k
            nc.sync.dma_start(
                of[1][:].rearrange("p (k c j) -> p c (k j)", k=NCORES, c=4), d_oi[:]
            )
            nc.vector.memset(c0[:], 0.0)
            nc.vector.memset(c1[:], 0.0)

            pid = nc.vector.partition_id()
            pid_pl = nc.gpsimd.partition_id()

            def lstm_pointwise(g_sb, cst, h_out):
                """g_sb [B, 4*HC] gate order i,f,g,o; updates cst, writes h_out [B,HC]."""
                it = tpool.tile([B, HC], fp32, tag="pw_it")
                ft = tpool.tile([B, HC], fp32, tag="pw_ft")
                gt = tpool.tile([B, HC], fp32, tag="pw_gt")
                ot = tpool.tile([B, HC], fp32, tag="pw_ot")
                nc.scalar.activation(it[:], g_sb[:, 0 * HC : 1 * HC], AF.Sigmoid)
                nc.scalar.activation(ft[:], g_sb[:, 1 * HC : 2 * HC], AF.Sigmoid)
                nc.scalar.activation(gt[:], g_sb[:, 2 * HC : 3 * HC], AF.Tanh)
                nc.scalar.activation(ot[:], g_sb[:, 3 * HC : 4 * HC], AF.Sigmoid)
                t1 = tpool.tile([B, HC], fp32, tag="pw_t1")
                nc.vector.tensor_mul(t1[:], ft[:], cst[:])
                nc.vector.tensor_mul(gt[:], it[:], gt[:])
                nc.vector.tensor_add(cst[:], t1[:], gt[:])
                tc_ = tpool.tile([B, HC], fp32, tag="pw_tc")
                nc.scalar.activation(tc_[:], cst[:], AF.Tanh)
                nc.vector.tensor_mul(h_out[:], ot[:], tc_[:])

            def exchange(t, kind, src_sb, width, dst_tile):
                """Broadcast my [P,width] chunk into slot pid of everyone's dst_tile."""
                rsem = rsems[2 * kind + (t % 2)]
                nc.gpsimd.remote_dma_broadcast(
                    dst_tile[:, bass.ts(pid_pl, width)],
                    src_sb,
                    rsem,
                    lsem,
                    rdests=RD,
                )
                nc.gpsimd.trigger_dma(count=None)
                with tc.tile_critical():
                    nc.vector.wait_ge(rsem, 16 * (t // 2 + 1))
                    nc.vector.tensor_copy(dst_tile[0:1, 0:1], dst_tile[0:1, 0:1])

            for t in range(n_steps):
                # ---- x load ----
                xt = xpool.tile([P, 4, B], fp32, tag="xt")
                nc.sync.dma_start(xt[:], d_xseq[t])

                # ---- gates0: K = [x(4) | o(4) | h0(8)] ----
                h0f_r = h0f[(t - 1) % 2]
                h1f_r = h1f[(t - 1) % 2]
                of_r = of[(t - 1) % 2]
                of_rv = of_r[:].rearrange("p (k c j) -> p c k j", k=NCORES, c=4)
                pg0 = ppool.tile([B, GC], fp32, tag="pg")
                for kt in range(16):
                    if kt < 4:
                        lhsT = xt[:, kt, :]
                    elif kt < 8:
                        lhsT = of_rv[:, kt - 4, :, :]
                    else:
                        lhsT = h0f_r[:, (kt - 8) * B : (kt - 7) * B]
                    nc.tensor.matmul(
                        pg0[:], lhsT, w0T[:, kt, :], start=(kt == 0), stop=(kt == 15)
                    )
                g0 = tpool.tile([B, GC], fp32, tag="g0")
                nc.vector.tensor_add(g0[:], pg0[:], b0[:])
                h0m = tpool.tile([B, HC], fp32, tag="h0m")
                lstm_pointwise(g0, c0, h0m)

                # ---- transpose h0m -> [HC, B], AG -> h0f ----
                pt0 = p1pool.tile([P, B], fp32, tag="ptr")
                nc.tensor.transpose(pt0[:], h0m[:], ident[0:B, 0:B])
                h0T = tpool.tile([P, B], fp32, tag="h0T")
                nc.vector.tensor_copy(h0T[:], pt0[:])

                exchange(t, 0, h0T[:], B, h0f[t % 2])

                # ---- gates1: K = [h0(8) | h1(8)] ----
                h0f_w = h0f[t % 2]
                pg1 = ppool.tile([B, GC], fp32, tag="pg")
                for kt in range(16):
                    lhsT = (
                        h0f_w[:, kt * B : (kt + 1) * B]
                        if kt < 8
                        else h1f_r[:, (kt - 8) * B : (kt - 7) * B]
                    )
                    nc.tensor.matmul(
                        pg1[:], lhsT, w1T[:, kt, :], start=(kt == 0), stop=(kt == 15)
                    )
                g1 = tpool.tile([B, GC], fp32, tag="g1")
                nc.vector.tensor_add(g1[:], pg1[:], b1[:])
                h1m = tpool.tile([B, HC], fp32, tag="h1m")
                lstm_pointwise(g1, c1, h1m)

                # ---- transpose h1m, AG -> h1f ----
                pt1 = p1pool.tile([P, B], fp32, tag="ptr")
                nc.tensor.transpose(pt1[:], h1m[:], ident[0:B, 0:B])
                h1T = tpool.tile([P, B], fp32, tag="h1T")
                nc.vector.tensor_copy(h1T[:], pt1[:])

                exchange(t, 1, h1T[:], B, h1f[t % 2])

                # ---- select my batch columns of h1 (query) ----
                h1f_wv = h1f[t % 2][:].rearrange("p (kc b) -> p kc b", kc=8)
                nc.vector.tensor_copy(h1my[:], h1f_wv[:, :, bass.ts(pid, BG)])

                # ---- scores: per-b matvec via tile_position packing ----
                psc = p1pool.tile([P, 2, S], fp32, tag="psc")
                nc.vector.memset(psc[:], 0.0)
                for j in range(BG):
                    half, row = j // 4, 32 * (j % 4)
                    for kt in range(8):
                        nc.tensor.matmul(
                            psc[row : row + 1, half, :],
                            h1my[:, kt, j : j + 1],
                            m1t[:, j, kt, :],
                            start=(kt == 0),
                            stop=(kt == 7),
                            tile_position=(0, row),
                        )
                # ---- softmax over the two halves (garbage rows are fine) ----
                a_sb = tpool.tile([P, 2, S], fp32, tag="a_sb")
                stat = tpool.tile([P, 4], fp32, tag="stat")
                for half in range(2):
                    nc.vector.tensor_add(
                        a_sb[:, half, :], psc[:, half, :], c1t[:, half, :]
                    )
                    nm = stat[:, 2 * half : 2 * half + 1]
                    nc.vector.tensor_reduce(
                        nm, a_sb[:, half, :], axis=AX.X, op=mybir.AluOpType.max,
                        negate=True,
                    )
                    nc.scalar.activation(a_sb[:, half, :], a_sb[:, half, :], AF.Exp, bias=nm)
                    sm = stat[:, 2 * half + 1 : 2 * half + 2]
                    nc.vector.tensor_reduce(
                        sm, a_sb[:, half, :], axis=AX.X, op=mybir.AluOpType.add
                    )
                    nc.vector.reciprocal(sm, sm)
                    nc.vector.tensor_scalar_mul(a_sb[:, half, :], a_sb[:, half, :], sm)

                # ---- transpose a -> columns; build block-diag lhsT ----
                paT = p1pool.tile([P, 2, S], fp32, tag="paT")
                nc.tensor.transpose(paT[:, 0, :], a_sb[:, 0, :], ident[:])
                nc.tensor.transpose(paT[:, 1, :], a_sb[:, 1, :], ident[:])
                abd = tpool.tile([P, BG, BG], fp32, tag="abd")
                nc.vector.memset(abd[:], 0.0)
                for j in range(BG):
                    col = 32 * (j % 4)
                    nc.vector.tensor_copy(
                        abd[:, j, j : j + 1], paT[:, j // 4, col : col + 1]
                    )

                # ---- z = blockdiag(a) @ M2stack + h1my.T @ W2h.T ----
                pz = p1pool.tile([BG, E], fp32, tag="pz")
                for j in range(BG):
                    nc.tensor.matmul(
                        pz[:], abd[:, j, :], m2s[:, j, :], start=(j == 0), stop=False
                    )
                for kt in range(8):
                    nc.tensor.matmul(
                        pz[:], h1my[:, kt, :], w2hT[:, kt, :], start=False,
                        stop=(kt == 7),
                    )
                zt = tpool.tile([BG, E], fp32, tag="zt")
                nc.vector.tensor_add(zt[:], pz[:], b2[:])
                o_sb = tpool.tile([BG, E], fp32, tag="o_sb")
                nc.scalar.activation(o_sb[:], zt[:], AF.Tanh)

                # ---- write output ----
                nc.sync.dma_start(d_out[t], o_sb[:])

                # ---- transpose o chunks -> [P, 4, BG], AG -> of ----
                poT = p1pool.tile([P, 4, BG], fp32, tag="poT")
                for cchunk in range(4):
                    nc.tensor.transpose(
                        poT[:, cchunk, :],
                        o_sb[:, cchunk * P : (cchunk + 1) * P],
                        ident[0:BG, 0:BG],
                    )
                oT = tpool.tile([P, 4 * BG], fp32, tag="oT")
                nc.vector.tensor_copy(oT[:].rearrange("p (c j) -> p c j", c=4), poT[:])

                exchange(t, 2, oT[:], 4 * BG, of[t % 2])

    nc.compile()
    return nc


def _host_prep(inputs: dict, n_steps: int):
    """Build per-core in_maps."""
    f32 = np.float32
    tgt = np.asarray(inputs["tgt_batch"])
    h_enc = np.asarray(inputs["h_encoder"], f32)
    emb = np.asarray(inputs["emb"], f32)
    out_init = np.asarray(inputs["output_init"], f32)
    hid_init = np.asarray(inputs["hidden_init"], f32)
    W_ih = np.asarray(inputs["W_ih"], f32)
    W_hh = np.asarray(inputs["W_hh"], f32)
    b_ih = np.asarray(inputs["b_ih"], f32)
    b_hh = np.asarray(inputs["b_hh"], f32)
    W1 = np.asarray(inputs["W1"], f32)
    b1v = np.asarray(inputs["b1"], f32)
    W2 = np.asarray(inputs["W2"], f32)
    b2v = np.asarray(inputs["b2"], f32)

    # x sequence, feature-major, folded [T, P, 4, B]
    xs = emb[tgt[:n_steps]]  # [T, B, E]
    xseq = np.ascontiguousarray(
        xs.transpose(0, 2, 1).reshape(n_steps, 4, P, B).transpose(0, 2, 1, 3)
    )

    # full o / h inits, feature-major folds
    oi = np.ascontiguousarray(out_init.T.reshape(4, P, B).transpose(1, 0, 2))
    h0i = np.ascontiguousarray(hid_init[0].T.reshape(8, P, B).transpose(1, 0, 2))
    h1i = np.ascontiguousarray(hid_init[1].T.reshape(8, P, B).transpose(1, 0, 2))

    # attention precompute (host for now; small fraction of FLOPs)
    # M1T[b] = (h_enc[b] @ W1).T  [H, S];  c1[b] = h_enc[b] @ b1  [S]
    # M2[b] = h_enc[b] @ W2v.T  [S, E]
    W2v, W2h = W2[:, :V], W2[:, V:]
    M1T = np.einsum("bsv,vh->bhs", h_enc, W1).astype(f32)  # [B, H, S]
    c1v = np.einsum("bsv,v->bs", h_enc, b1v).astype(f32)  # [B, S]
    M2 = np.einsum("bsv,ev->bse", h_enc, W2v).astype(f32)  # [B, S, E]

    in_maps = []
    for k in range(NCORES):
        rows = np.concatenate([np.arange(g * H + k * HC, g * H + (k + 1) * HC) for g in range(4)])
        W0c = np.concatenate([W_ih[0], W_hh[0]], axis=1)[rows]  # [GC, 2048]
        W1c = np.concatenate([W_ih[1], W_hh[1]], axis=1)[rows]
        w0T = np.ascontiguousarray(W0c.T.reshape(16, P, GC))
        w1T = np.ascontiguousarray(W1c.T.reshape(16, P, GC))
        b0c = np.broadcast_to((b_ih[0] + b_hh[0])[rows], (B, GC)).copy()
        b1c = np.broadcast_to((b_ih[1] + b_hh[1])[rows], (B, GC)).copy()

        bs = np.arange(k * BG, (k + 1) * BG)
        m1tc = np.ascontiguousarray(M1T[bs].reshape(BG, 8, P, S))
        m2sc = np.ascontiguousarray(M2[bs])  # [BG, S, E] (S=P)
        c1tc = np.zeros((2, P, S), f32)
        for j in range(BG):
            c1tc[j // 4, 32 * (j % 4), :] = c1v[bs[j]]
        w2hT = np.ascontiguousarray(W2h.T.reshape(8, P, E))
        b2c = np.broadcast_to(b2v, (BG, E)).copy()

        in_maps.append(
            {
                "xseq": xseq,
                "w0T": w0T,
                "w1T": w1T,
                "b0": b0c,
                "b1": b1c,
                "m1t": m1tc,
                "c1t": c1tc,
                "m2s": m2sc,
                "w2hT": w2hT,
                "b2": b2c,
                "h0i": h0i,
                "h1i": h1i,
                "oi": oi,
            }
        )
    return in_maps


def run(inputs: dict, n_steps: int = T, trace: bool = False):
    global _COMPILED
    from concourse.bass_utils import run_bass_kernel_spmd

    if _COMPILED is None or _COMPILED[1] != n_steps:
        _COMPILED = (_build(n_steps), n_steps)
    nc = _COMPILED[0]
    in_maps = _host_prep(inputs, n_steps)
    res = run_bass_kernel_spmd(
        nc, in_maps, core_ids=list(range(NCORES)), trace=trace
    )
    outs = [res.results[k]["out"] for k in range(NCORES)]  # [T, BG, E] each
    full = np.concatenate(outs, axis=1)  # [T, B, E]
    return np.ascontiguousarray(full.transpose(1, 0, 2)), res  # [B, T, E]


def kernel(**inputs) -> np.ndarray:
    out, _ = run(inputs, T)
    return out.astype(np.float32)


# revision 13
# speedup vs baseline: 1.2429x; 1.2429x over previous
"""AttentionDecoder Trainium2 kernel: 8-way model-parallel LSTM+attention decoder.

Strategy:
  - Weights sharded 8 ways over the gate/hidden dims, SBUF-resident.
  - Activations feature-major [feat, batch]; matmuls are activation-stationary
    (lhsT = activation [K=feat, M=batch], rhs = weight.T [K=feat, N=out_feats]).
  - Per timestep: 3 AllGathers (h0, h1, o) across the 8 cores.
  - Attention refactored: M1T[b] = (h_enc[b] @ W1).T and M2[b] = h_enc[b] @ W2v.T
    are precomputed (hoists h_enc out of the sequential loop), so per step
    scores[b] = M1T[b].T @ h1[:,b] + c1[b] and
    z[b] = a[b] @ M2[b] + W2h @ h1[:,b] + b2,  o = tanh(z).
  - Per-core batch shard for attention: core k owns batch 8k..8k+7.
"""

import os
import warnings

warnings.filterwarnings("ignore")

import numpy as np

VOCAB, E, H, L, B, T, S, V = 32000, 512, 1024, 2, 64, 64, 128, 1024
NCORES = 8
P = 128
BG = B // NCORES  # 8 batch per core for attention
HC = H // NCORES  # 128 hidden feats per core
GC = 4 * HC  # 512 gate rows per core

REMOTE_MODE = int(os.environ.get("DEC_REMOTE", "0"))
USE_REMOTE = REMOTE_MODE >= 1

_COMPILED = None


def _build(n_steps: int):
    import concourse.bass as bass
    import concourse.bacc as bacc
    import concourse.mybir as mybir
    import concourse.tile as tile
    from concourse import masks

    fp32 = mybir.dt.float32
    AF = mybir.ActivationFunctionType
    AX = mybir.AxisListType

    nc = bacc.Bacc(
        "TRN2",
        target_bir_lowering=False,
        debug=False,
        num_devices=NCORES,
        monotonic_sem_count=12,
    )
    rsems = [nc.monotonic_semaphore(i).sem() for i in range(6)]  # h0e,h0o,h1e,h1o,oe,oo
    lsems = [nc.monotonic_semaphore(6 + i).sem() for i in range(6)]
    import os as _os
    _rm = int(_os.environ.get("DEC_REMOTE", "0"))
    RD = [(0, d) for d in range(NCORES)]
    if _rm == 2:
        RD = [None] + [(0, d) for d in range(1, NCORES)]
    RSEM_PER_ROUND = 14 if _rm == 2 else 16

    # ---- DRAM parameters (per-core data) ----
    d_xseq = nc.dram_tensor("xseq", [n_steps, P, 4, B], fp32, kind="ExternalInput")
    d_w0T = nc.dram_tensor("w0T", [16, P, GC], fp32, kind="ExternalInput")
    d_w1T = nc.dram_tensor("w1T", [16, P, GC], fp32, kind="ExternalInput")
    d_b0 = nc.dram_tensor("b0", [B, GC], fp32, kind="ExternalInput")
    d_b1 = nc.dram_tensor("b1", [B, GC], fp32, kind="ExternalInput")
    d_m1t = nc.dram_tensor("m1t", [BG, 8, P, S], fp32, kind="ExternalInput")
    d_c1t = nc.dram_tensor("c1t", [2, P, S], fp32, kind="ExternalInput")
    d_m2s = nc.dram_tensor("m2s", [BG, P, E], fp32, kind="ExternalInput")
    d_w2hT = nc.dram_tensor("w2hT", [8, P, E], fp32, kind="ExternalInput")
    d_b2 = nc.dram_tensor("b2", [BG, E], fp32, kind="ExternalInput")
    d_h0i = nc.dram_tensor("h0i", [P, 8, B], fp32, kind="ExternalInput")
    d_h1i = nc.dram_tensor("h1i", [P, 8, B], fp32, kind="ExternalInput")
    d_oi = nc.dram_tensor("oi", [P, NCORES * 4 * BG], fp32, kind="ExternalInput")
    d_out = nc.dram_tensor("out", [n_steps, BG, E], fp32, kind="ExternalOutput")

    with tile.TileContext(nc) as tc:
        import contextlib

        ctx = contextlib.ExitStack()
        with ctx:
            wpool = ctx.enter_context(tc.tile_pool(name="weights", bufs=1))
            spool = ctx.enter_context(tc.tile_pool(name="state", bufs=1))
            xpool = ctx.enter_context(tc.tile_pool(name="x", bufs=2))
            tpool = ctx.enter_context(tc.tile_pool(name="tmp", bufs=2))
            ppool = ctx.enter_context(tc.tile_pool(name="psum", bufs=2, space="PSUM"))
            p1pool = ctx.enter_context(tc.tile_pool(name="psum1", bufs=1, space="PSUM"))
            dpool = ctx.enter_context(tc.tile_pool(name="dram", bufs=2, space="DRAM"))

            # ---- persistent SBUF tiles ----
            w0T = wpool.tile([P, 16, GC], fp32, tag="w0T")
            w1T = wpool.tile([P, 16, GC], fp32, tag="w1T")
            b0 = wpool.tile([B, GC], fp32, tag="b0")
            b1 = wpool.tile([B, GC], fp32, tag="b1")
            m1t = wpool.tile([P, BG, 8, S], fp32, tag="m1t")
            c1t = wpool.tile([P, 2, S], fp32, tag="c1t")
            m2s = wpool.tile([P, BG, E], fp32, tag="m2s")
            w2hT = wpool.tile([P, 8, E], fp32, tag="w2hT")
            b2 = wpool.tile([BG, E], fp32, tag="b2")
            ident = wpool.tile([P, P], fp32, tag="ident")

            h0f = [spool.tile([P, NCORES * B], fp32, tag=f"h0f{i}", name=f"h0f{i}") for i in range(2)]
            h1f = [spool.tile([P, NCORES * B], fp32, tag=f"h1f{i}", name=f"h1f{i}") for i in range(2)]
            of = [spool.tile([P, NCORES * 4 * BG], fp32, tag=f"of{i}", name=f"of{i}") for i in range(2)]
            c0 = spool.tile([B, HC], fp32, tag="c0")  # cell states, batch-major
            c1 = spool.tile([B, HC], fp32, tag="c1")
            h1my = spool.tile([P, 8, BG], fp32, tag="h1my")

            # ---- prologue loads ----
            for kt in range(16):
                nc.sync.dma_start(w0T[:, kt, :], d_w0T[kt])
                nc.sync.dma_start(w1T[:, kt, :], d_w1T[kt])
            nc.sync.dma_start(b0[:], d_b0[:])
            nc.sync.dma_start(b1[:], d_b1[:])
            for j in range(BG):
                for kt in range(8):
                    nc.sync.dma_start(m1t[:, j, kt, :], d_m1t[j, kt])
                nc.sync.dma_start(m2s[:, j, :], d_m2s[j])
            nc.sync.dma_start(c1t[:, 0, :], d_c1t[0])
            nc.sync.dma_start(c1t[:, 1, :], d_c1t[1])
            for kt in range(8):
                nc.sync.dma_start(w2hT[:, kt, :], d_w2hT[kt])
            nc.sync.dma_start(b2[:], d_b2[:])
            masks.make_identity(nc, ident[:])
            nc.sync.dma_start(h0f[1][:].rearrange("p (kc b) -> p kc b", kc=8), d_h0i[:])
            nc.sync.dma_start(h1f[1][:].rearrange("p (kc b) -> p kc b", kc=8), d_h1i[:])
            # of init: slot k holds o columns [c, j] for b-group k
            nc.sync.dma_start(of[1][:], d_oi[:])
            nc.vector.memset(c0[:], 0.0)
            nc.vector.memset(c1[:], 0.0)

            pid = nc.vector.partition_id()
            pid_pl = nc.gpsimd.partition_id()

            def lstm_pointwise(g_sb, cst, h_out):
                """g_sb [B, 4*HC] gate order i,f,g,o; updates cst, writes h_out [B,HC]."""
                it = tpool.tile([B, HC], fp32, tag="pw_it")
                ft = tpool.tile([B, HC], fp32, tag="pw_ft")
                gt = tpool.tile([B, HC], fp32, tag="pw_gt")
                ot = tpool.tile([B, HC], fp32, tag="pw_ot")
                nc.scalar.activation(it[:], g_sb[:, 0 * HC : 1 * HC], AF.Sigmoid)
                nc.scalar.activation(ft[:], g_sb[:, 1 * HC : 2 * HC], AF.Sigmoid)
                nc.scalar.activation(gt[:], g_sb[:, 2 * HC : 3 * HC], AF.Tanh)
                nc.scalar.activation(ot[:], g_sb[:, 3 * HC : 4 * HC], AF.Sigmoid)
                t1 = tpool.tile([B, HC], fp32, tag="pw_t1")
                nc.vector.tensor_mul(t1[:], ft[:], cst[:])
                nc.vector.tensor_mul(gt[:], it[:], gt[:])
                nc.vector.tensor_add(cst[:], t1[:], gt[:])
                tc_ = tpool.tile([B, HC], fp32, tag="pw_tc")
                nc.scalar.activation(tc_[:], cst[:], AF.Tanh)
                nc.vector.tensor_mul(h_out[:], ot[:], tc_[:])

            def evict_src(t, kind, dst_ap, src_ap):
                if t >= 2 and USE_REMOTE:
                    with tc.tile_critical():
                        nc.vector.wait_ge(lsems[2 * kind + (t % 2)], 16 * (t // 2))
                        nc.vector.tensor_copy(dst_ap, src_ap)
                else:
                    nc.vector.tensor_copy(dst_ap, src_ap)

            def exchange(t, kind, src_sb, width, dst_tile):
                """Broadcast my [P,width] chunk into slot pid of everyone's dst_tile."""
                if not USE_REMOTE:
                    bi = dpool.tile([P, width], fp32, tag=f"agi{kind}", name=f"agi{kind}")
                    bo = dpool.tile(
                        [P * NCORES, width], fp32, tag=f"ago{kind}", name=f"ago{kind}"
                    )
                    nc.sync.dma_start(bi[:], src_sb)
                    nc.gpsimd.collective_compute(
                        "AllGather",
                        mybir.AluOpType.bypass,
                        replica_groups=[list(range(NCORES))],
                        ins=[bi.opt()],
                        outs=[bo.opt()],
                    )
                    nc.sync.dma_start(
                        dst_tile[:].rearrange("p (k w) -> p k w", k=NCORES),
                        bo[:].rearrange("(k p) w -> p k w", p=P),
                    )
                    return
                rsem = rsems[2 * kind + (t % 2)]
                nc.gpsimd.remote_dma_broadcast(
                    dst_tile[:, bass.ts(pid_pl, width)],
                    src_sb,
                    rsem,
                    lsems[2 * kind + (t % 2)],
                    rdests=RD,
                )
                nc.gpsimd.trigger_dma(count=None)
                if RSEM_PER_ROUND == 14:
                    # self slot not broadcast; copy locally
                    nc.vector.tensor_copy(
                        dst_tile[:, bass.ts(pid, width)], src_sb
                    )
                with tc.tile_critical():
                    nc.vector.wait_ge(rsem, RSEM_PER_ROUND * (t // 2 + 1))
                    nc.vector.tensor_copy(dst_tile[0:1, 0:1], dst_tile[0:1, 0:1])

            for t in range(n_steps):
                # ---- x load ----
                xt = xpool.tile([P, 4, B], fp32, tag="xt")
                nc.sync.dma_start(xt[:], d_xseq[t])

                # ---- gates0: K = [x(4) | o(4) | h0(8)] ----
                h0f_r = h0f[(t - 1) % 2]
                h1f_r = h1f[(t - 1) % 2]
                of_r = of[(t - 1) % 2]
                of_rv = of_r[:].rearrange("p (k c j) -> p c k j", k=NCORES, c=4)
                o4 = tpool.tile([P, 4, B], fp32, tag="o4")
                nc.vector.tensor_copy(
                    o4[:].rearrange("p c (k j) -> p c k j", k=NCORES), of_rv
                )
                pg0 = ppool.tile([B, GC], fp32, tag="pg")
                order0 = [0, 1, 2, 3] + [8, 9, 10, 11, 12, 13, 14, 15] + [4, 5, 6, 7]
                for i, kt in enumerate(order0):
                    if kt < 4:
                        lhsT = xt[:, kt, :]
                    elif kt < 8:
                        lhsT = o4[:, kt - 4, :]
                    else:
                        lhsT = h0f_r[:, (kt - 8) * B : (kt - 7) * B]
                    nc.tensor.matmul(
                        pg0[:], lhsT, w0T[:, kt, :], start=(i == 0), stop=(i == 15)
                    )
                g0 = tpool.tile([B, GC], fp32, tag="g0")
                nc.vector.tensor_add(g0[:], pg0[:], b0[:])
                h0m = tpool.tile([B, HC], fp32, tag="h0m")
                lstm_pointwise(g0, c0, h0m)

                # ---- transpose h0m -> [HC, B], AG -> h0f ----
                pt0 = p1pool.tile([P, B], fp32, tag="ptr")
                nc.tensor.transpose(pt0[:], h0m[:], ident[0:B, 0:B])
                h0T = tpool.tile([P, B], fp32, tag="h0T")
                evict_src(t, 0, h0T[:], pt0[:])

                exchange(t, 0, h0T[:], B, h0f[t % 2])

                # ---- gates1: K = [h0(8) | h1(8)] ----
                h0f_w = h0f[t % 2]
                pg1 = ppool.tile([B, GC], fp32, tag="pg")
                order1 = [8, 9, 10, 11, 12, 13, 14, 15] + [0, 1, 2, 3, 4, 5, 6, 7]
                for i, kt in enumerate(order1):
                    lhsT = (
                        h0f_w[:, kt * B : (kt + 1) * B]
                        if kt < 8
                        else h1f_r[:, (kt - 8) * B : (kt - 7) * B]
                    )
                    nc.tensor.matmul(
                        pg1[:], lhsT, w1T[:, kt, :], start=(i == 0), stop=(i == 15)
                    )
                g1 = tpool.tile([B, GC], fp32, tag="g1")
                nc.vector.tensor_add(g1[:], pg1[:], b1[:])
                h1m = tpool.tile([B, HC], fp32, tag="h1m")
                lstm_pointwise(g1, c1, h1m)

                # ---- transpose h1m, AG -> h1f ----
                pt1 = p1pool.tile([P, B], fp32, tag="ptr")
                nc.tensor.transpose(pt1[:], h1m[:], ident[0:B, 0:B])
                h1T = tpool.tile([P, B], fp32, tag="h1T")
                evict_src(t, 1, h1T[:], pt1[:])

                exchange(t, 1, h1T[:], B, h1f[t % 2])

                # ---- select my batch columns of h1 (query) ----
                h1f_wv = h1f[t % 2][:].rearrange("p (kc b) -> p kc b", kc=8)
                nc.vector.tensor_copy(h1my[:], h1f_wv[:, :, bass.ts(pid, BG)])

                # ---- scores: per-b matvec via tile_position packing ----
                psc = p1pool.tile([P, 2, S], fp32, tag="psc")
                nc.vector.memset(psc[:], 0.0)
                for j in range(BG):
                    half, row = j // 4, 32 * (j % 4)
                    for kt in range(8):
                        nc.tensor.matmul(
                            psc[row : row + 1, half, :],
                            h1my[:, kt, j : j + 1],
                            m1t[:, j, kt, :],
                            start=(kt == 0),
                            stop=(kt == 7),
                            tile_position=(0, row),
                        )
                # ---- softmax over the two halves (garbage rows are fine) ----
                a_sb = tpool.tile([P, 2, S], fp32, tag="a_sb")
                stat = tpool.tile([P, 4], fp32, tag="stat")
                for half in range(2):
                    nc.vector.tensor_add(
                        a_sb[:, half, :], psc[:, half, :], c1t[:, half, :]
                    )
                    nm = stat[:, 2 * half : 2 * half + 1]
                    nc.vector.tensor_reduce(
                        nm, a_sb[:, half, :], axis=AX.X, op=mybir.AluOpType.max,
                        negate=True,
                    )
                    nc.scalar.activation(a_sb[:, half, :], a_sb[:, half, :], AF.Exp, bias=nm)
                    sm = stat[:, 2 * half + 1 : 2 * half + 2]
                    nc.vector.tensor_reduce(
                        sm, a_sb[:, half, :], axis=AX.X, op=mybir.AluOpType.add
                    )
                    nc.vector.reciprocal(sm, sm)
                    nc.vector.tensor_scalar_mul(a_sb[:, half, :], a_sb[:, half, :], sm)

                # ---- transpose a -> columns; build block-diag lhsT ----
                paT = p1pool.tile([P, 2, S], fp32, tag="paT")
                nc.tensor.transpose(paT[:, 0, :], a_sb[:, 0, :], ident[:])
                nc.tensor.transpose(paT[:, 1, :], a_sb[:, 1, :], ident[:])
                abd = tpool.tile([P, BG, BG], fp32, tag="abd")
                nc.vector.memset(abd[:], 0.0)
                for j in range(BG):
                    col = 32 * (j % 4)
                    nc.vector.tensor_copy(
                        abd[:, j, j : j + 1], paT[:, j // 4, col : col + 1]
                    )

                # ---- z = blockdiag(a) @ M2stack + h1my.T @ W2h.T ----
                pz = p1pool.tile([BG, E], fp32, tag="pz")
                for j in range(BG):
                    nc.tensor.matmul(
                        pz[:], abd[:, j, :], m2s[:, j, :], start=(j == 0), stop=False
                    )
                for kt in range(8):
                    nc.tensor.matmul(
                        pz[:], h1my[:, kt, :], w2hT[:, kt, :], start=False,
                        stop=(kt == 7),
                    )
                zt = tpool.tile([BG, E], fp32, tag="zt")
                nc.vector.tensor_add(zt[:], pz[:], b2[:])
                o_sb = tpool.tile([BG, E], fp32, tag="o_sb")
                nc.scalar.activation(o_sb[:], zt[:], AF.Tanh)

                # ---- write output ----
                nc.sync.dma_start(d_out[t], o_sb[:])

                # ---- transpose o chunks -> [P, 4, BG], AG -> of ----
                poT = p1pool.tile([P, 4, BG], fp32, tag="poT")
                for cchunk in range(4):
                    nc.tensor.transpose(
                        poT[:, cchunk, :],
                        o_sb[:, cchunk * P : (cchunk + 1) * P],
                        ident[0:BG, 0:BG],
                    )
                oT = tpool.tile([P, 4 * BG], fp32, tag="oT")
                evict_src(t, 2, oT[:].rearrange("p (c j) -> p c j", c=4), poT[:])

                exchange(t, 2, oT[:], 4 * BG, of[t % 2])

    nc.compile()
    return nc


def _host_prep(inputs: dict, n_steps: int):
    """Build per-core in_maps."""
    f32 = np.float32
    tgt = np.asarray(inputs["tgt_batch"])
    h_enc = np.asarray(inputs["h_encoder"], f32)
    emb = np.asarray(inputs["emb"], f32)
    out_init = np.asarray(inputs["output_init"], f32)
    hid_init = np.asarray(inputs["hidden_init"], f32)
    W_ih = np.asarray(inputs["W_ih"], f32)
    W_hh = np.asarray(inputs["W_hh"], f32)
    b_ih = np.asarray(inputs["b_ih"], f32)
    b_hh = np.asarray(inputs["b_hh"], f32)
    W1 = np.asarray(inputs["W1"], f32)
    b1v = np.asarray(inputs["b1"], f32)
    W2 = np.asarray(inputs["W2"], f32)
    b2v = np.asarray(inputs["b2"], f32)

    # x sequence, feature-major, folded [T, P, 4, B]
    xs = emb[tgt[:n_steps]]  # [T, B, E]
    xseq = np.ascontiguousarray(
        xs.transpose(0, 2, 1).reshape(n_steps, 4, P, B).transpose(0, 2, 1, 3)
    )

    # full o / h inits, feature-major folds
    # oi[p, (k, c, j)] = o[c*128+p, 8k+j]
    oi4 = out_init.T.reshape(4, P, NCORES, 8)  # [c, p, k, j]
    oi = np.ascontiguousarray(oi4.transpose(1, 2, 0, 3).reshape(P, NCORES * 4 * 8))
    h0i = np.ascontiguousarray(hid_init[0].T.reshape(8, P, B).transpose(1, 0, 2))
    h1i = np.ascontiguousarray(hid_init[1].T.reshape(8, P, B).transpose(1, 0, 2))

    # attention precompute (host for now; small fraction of FLOPs)
    # M1T[b] = (h_enc[b] @ W1).T  [H, S];  c1[b] = h_enc[b] @ b1  [S]
    # M2[b] = h_enc[b] @ W2v.T  [S, E]
    W2v, W2h = W2[:, :V], W2[:, V:]
    M1T = np.einsum("bsv,vh->bhs", h_enc, W1).astype(f32)  # [B, H, S]
    c1v = np.einsum("bsv,v->bs", h_enc, b1v).astype(f32)  # [B, S]
    M2 = np.einsum("bsv,ev->bse", h_enc, W2v).astype(f32)  # [B, S, E]

    in_maps = []
    for k in range(NCORES):
        rows = np.concatenate([np.arange(g * H + k * HC, g * H + (k + 1) * HC) for g in range(4)])
        W0c = np.concatenate([W_ih[0], W_hh[0]], axis=1)[rows]  # [GC, 2048]
        W1c = np.concatenate([W_ih[1], W_hh[1]], axis=1)[rows]
        w0T = np.ascontiguousarray(W0c.T.reshape(16, P, GC))
        w1T = np.ascontiguousarray(W1c.T.reshape(16, P, GC))
        b0c = np.broadcast_to((b_ih[0] + b_hh[0])[rows], (B, GC)).copy()
        b1c = np.broadcast_to((b_ih[1] + b_hh[1])[rows], (B, GC)).copy()

        bs = np.arange(k * BG, (k + 1) * BG)
        m1tc = np.ascontiguousarray(M1T[bs].reshape(BG, 8, P, S))
        m2sc = np.ascontiguousarray(M2[bs])  # [BG, S, E] (S=P)
        c1tc = np.zeros((2, P, S), f32)
        for j in range(BG):
            c1tc[j // 4, 32 * (j % 4), :] = c1v[bs[j]]
        w2hT = np.ascontiguousarray(W2h.T.reshape(8, P, E))
        b2c = np.broadcast_to(b2v, (BG, E)).copy()

        in_maps.append(
            {
                "xseq": xseq,
                "w0T": w0T,
                "w1T": w1T,
                "b0": b0c,
                "b1": b1c,
                "m1t": m1tc,
                "c1t": c1tc,
                "m2s": m2sc,
                "w2hT": w2hT,
                "b2": b2c,
                "h0i": h0i,
                "h1i": h1i,
                "oi": oi,
            }
        )
    return in_maps


def run(inputs: dict, n_steps: int = T, trace: bool = False):
    global _COMPILED
    from concourse.bass_utils import run_bass_kernel_spmd

    if _COMPILED is None or _COMPILED[1] != n_steps:
        _COMPILED = (_build(n_steps), n_steps)
    nc = _COMPILED[0]
    in_maps = _host_prep(inputs, n_steps)
    res = run_bass_kernel_spmd(
        nc, in_maps, core_ids=list(range(NCORES)), trace=trace
    )
    outs = [res.results[k]["out"] for k in range(NCORES)]  # [T, BG, E] each
    full = np.concatenate(outs, axis=1)  # [T, B, E]
    return np.ascontiguousarray(full.transpose(1, 0, 2)), res  # [B, T, E]


def kernel(**inputs) -> np.ndarray:
    out, _ = run(inputs, T)
    return out.astype(np.float32)
